# revision 15
# baseline (speedup 1.0000x reference)
"""Trainium2 Bass kernel for nn_Attention_61229053772048 (dual-softmax linear attention).

Sharding: data-parallel over batch B=8 across 8 NeuronCores (one batch per core,
no collectives). Each core computes, for its batch x_b (4096, 256):

  K = x Wk^T, Q = x Wq^T, V = x Wv^T              (raw-reshape semantics:
  r-layout M_r (256, 4096) = row-major view of M (4096, 256))
  key softmax over the 4096 axis of K_r, query softmax over 32-row head groups
  of Q_r, context = Ksm_h @ V_h^T per head (32x32), attended = ctx^T @ Qsm,
  proj = Wp @ attended + bp + x_r, out = raw reshape back to (4096, 256).

v3 layout choices (from profile iteration):
  - all big matmuls in bf16 (fp32 matmuls run LOW_HIGH = 2x passes)
  - x is cast to bf16, bounced through DRAM, and transposed via the DMA XBAR
    into a q-major xT2 (128c, 2cc, 16q, 256r) so every matmul operand is
    contiguous (strided rhs reads measured 6x slower)
  - K/V projections as N=512 matmuls; Sk folded into the context matmul via a
    ones column embedded in the V tile (rhs width 129)
  - phase 2 streams 8 blocks of 2 q-columns: Q proj -> exp -> Sq/attended
    (block-diag lhsT, N=512) -> fast-reciprocal divide -> out proj -> +bias,
    +residual (gpsimd) -> DMA out
"""

import sys

sys.path.insert(0, "/opt/trn_rl_repo")

import numpy as np

import concourse.bass as bass
import concourse.bacc as bacc_mod
import concourse.tile as tile
from concourse import mybir
from concourse.bass_utils import run_bass_kernel_spmd
from concourse.masks import make_identity

F32 = mybir.dt.float32
BF16 = mybir.dt.bfloat16
Exp = mybir.ActivationFunctionType.Exp

N, C, P = 4096, 256, 128
NH, HD, Q16 = 8, 32, 16
NCORES = 8

_CACHE = {}


def _build_program():
    nc = bacc_mod.Bacc(None, target_bir_lowering=False, debug=False)
    x_e = nc.declare_dram_parameter("x", [N, C], F32, isOutput=False)
    wq_e = nc.declare_dram_parameter("Wq", [C, C], F32, isOutput=False)
    wk_e = nc.declare_dram_parameter("Wk", [C, C], F32, isOutput=False)
    wv_e = nc.declare_dram_parameter("Wv", [C, C], F32, isOutput=False)
    wp_e = nc.declare_dram_parameter("Wp", [C, C], F32, isOutput=False)
    bp_e = nc.declare_dram_parameter("bp", [C], F32, isOutput=False)
    out_e = nc.declare_dram_parameter("out", [N, C], F32, isOutput=True)

    with tile.TileContext(nc) as tc:
        _body(tc, x_e, wq_e, wk_e, wv_e, wp_e, bp_e, out_e)
    nc.compile()
    return nc


def _body(tc, x_e, wq_e, wk_e, wv_e, wp_e, bp_e, out_e):
    nc = tc.nc
    from contextlib import ExitStack

    ctx = ExitStack()
    consts = ctx.enter_context(tc.tile_pool(name="consts", bufs=1))
    wstage = ctx.enter_context(tc.tile_pool(name="wstage", bufs=2))
    xstage = ctx.enter_context(tc.tile_pool(name="xstage", bufs=3))
    bigs = ctx.enter_context(tc.tile_pool(name="bigs", bufs=1))
    p2pool = ctx.enter_context(tc.tile_pool(name="p2", bufs=3))
    ypool = ctx.enter_context(tc.tile_pool(name="y", bufs=4))
    dram = ctx.enter_context(tc.tile_pool(name="dram", bufs=1, space="DRAM"))
    psumA = ctx.enter_context(tc.tile_pool(name="psumA", bufs=4, space="PSUM"))
    psumT = ctx.enter_context(tc.tile_pool(name="psumT", bufs=2, space="PSUM"))

    # ---- constants ----
    identity = consts.tile([P, P], BF16)
    make_identity(nc, identity)

    blockones = consts.tile([P, P], BF16)
    nc.vector.memset(blockones, 0.0)
    for k in range(4):
        nc.vector.memset(blockones[32 * k : 32 * k + 32, 32 * k : 32 * k + 32], 1.0)

    bp_sb = consts.tile([P, 2], F32)
    nc.sync.dma_start(out=bp_sb, in_=bp_e.rearrange("(cc p) -> p cc", p=P))

    # ---- weight transposes: wt[p, cc, o] = W[o, 128*cc + p]  (bf16) ----
    wts = {}
    for name, w_e in (("q", wq_e), ("k", wk_e), ("v", wv_e), ("p", wp_e)):
        wn = wstage.tile([P, 2, C], F32, tag="wn")
        w_v = w_e.rearrange("(oc p) c -> p oc c", p=P)
        for oc in range(2):
            nc.sync.dma_start(out=wn[:, oc, :], in_=w_v[:, oc, :])
        wnb = wstage.tile([P, 2, C], BF16, tag="wnb")
        for oc in range(2):
            nc.vector.tensor_copy(out=wnb[:, oc, :], in_=wn[:, oc, :])
        wt = consts.tile([P, 2, C], BF16, tag=f"wt_{name}")
        for cc in range(2):
            for oc in range(2):
                ps = psumT.tile([P, P], BF16, tag="tw")
                nc.tensor.transpose(ps, wnb[:, oc, 128 * cc : 128 * cc + 128], identity)
                nc.vector.tensor_copy(out=wt[:, cc, 128 * oc : 128 * oc + 128], in_=ps)
        wts[name] = wt

    # ---- x -> bf16 -> DRAM bounce -> XBAR transpose into q-major xT2 ----
    # xT2[p, cc, q, r] = x[16r + q, 128cc + p]  (bf16)
    xbf = dram.tile([N, C], BF16)
    x_nt = x_e.rearrange("(nt p) c -> p nt c", p=P)  # (128, 32, 256)
    xbf_nt = xbf.rearrange("(nt p) c -> p nt c", p=P)
    for grp in range(8):
        xs = xstage.tile([P, 4, C], F32, tag="xs")
        nc.scalar.dma_start(out=xs, in_=x_nt[:, 4 * grp : 4 * grp + 4, :])
        xsb = xstage.tile([P, 4, C], BF16, tag="xsb")
        nc.vector.tensor_copy(out=xsb, in_=xs)
        nc.gpsimd.dma_start(out=xbf_nt[:, 4 * grp : 4 * grp + 4, :], in_=xsb)

    xT2 = bigs.tile([P, 2, Q16, C], BF16, tag="xT2")
    xbf_q = xbf.rearrange("(r q) c -> q r c", q=Q16)  # row 16r+q
    for cc in range(2):
        for q in range(Q16):
            nc.sync.dma_start_transpose(
                xT2[:, cc, q, :], xbf_q[q, :, 128 * cc : 128 * cc + 128]
            )

    # ---- K and V in n-part layout: chunk t = 2q + cj holds partitions c-half ----
    # dst[p, t, r] = sum_{c'} W[128*cj + p, c'] * x[16r + q, c']
    # Vb free layout per chunk: [g0 V(128) | 1.0 | g1 V(128) | 1.0] (129*2)
    # Q projections interleaved here so PE has work while K/V epilogues drain.
    expK = bigs.tile([P, 32, C], BF16, tag="expK")
    Vb = bigs.tile([P, 32, 258], BF16, tag="Vb")
    Vb_v = Vb.rearrange("p t (g x) -> p t g x", g=2)
    nc.vector.memset(Vb_v[:, :, :, 128], 1.0)
    expQall = bigs.tile([P, Q16, 2, C], BF16, tag="expQall")
    for cj in range(2):
        for qp in range(8):
            for wt, do_exp in ((wts["k"], True), (wts["v"], False)):
                ps = psumA.tile([P, 2, C], F32, tag="mm", name="kvps")
                for cp in range(2):
                    nc.tensor.matmul(
                        ps,
                        lhsT=wt[:, cp, 128 * cj : 128 * cj + 128],
                        rhs=xT2[:, cp, 2 * qp : 2 * qp + 2, :],
                        start=(cp == 0),
                        stop=(cp == 1),
                    )
                t0 = 4 * qp + cj  # chunks t0 and t0 + 2
                if do_exp:
                    nc.scalar.activation(
                        out=expK[:, t0 : t0 + 3 : 2, :], in_=ps, func=Exp
                    )
                else:
                    nc.vector.tensor_copy(
                        out=Vb_v[:, t0 : t0 + 3 : 2, :, 0:128],
                        in_=ps.rearrange("p two (g e) -> p two g e", g=2),
                    )
            for qi in range(2):
                q = 2 * qp + qi
                rc = cj
                qp_full = psumA.tile([P, 2, C], F32, tag="mm", name="qp_full")
                qp_ps = qp_full[:, 0, :]
                for cp in range(2):
                    nc.tensor.matmul(
                        qp_ps,
                        lhsT=xT2[:, cp, q, 128 * rc : 128 * rc + 128],
                        rhs=wts["q"][:, cp, :],
                        start=(cp == 0),
                        stop=(cp == 1),
                    )
                nc.scalar.activation(out=expQall[:, q, rc, :], in_=qp_ps, func=Exp)

    # ---- context + Sk (ones column) per 128-r-group, contraction over n ----
    # ctxp[d, e] = sum_n expK[n, 128g+d] * V[n, 128g+e];  col 128 = Sk
    ctx_sb = consts.tile([P, 2, P], F32, tag="ctx")
    recip_sk = consts.tile([P, 2], F32, tag="rsk")
    for g in range(2):
        ctxp = psumT.tile([P, 132], F32, tag="tc")
        for t in range(32):
            nc.tensor.matmul(
                ctxp[:, :129],
                lhsT=expK[:, t, 128 * g : 128 * g + 128],
                rhs=Vb[:, t, 129 * g : 129 * g + 129],
                start=(t == 0),
                stop=(t == 31),
            )
        nc.vector.reciprocal_approx_fast(
            out=recip_sk[:, g : g + 1], in_=ctxp[:, 128:129]
        )
        nc.vector.tensor_scalar_mul(
            out=ctx_sb[:, g, :], in0=ctxp[:, 0:128], scalar1=recip_sk[:, g : g + 1]
        )

    # ---- block-diagonal context for the attended matmul ----
    bdctx = consts.tile([P, 2, P], BF16, tag="bdctx")
    nc.vector.memset(bdctx, 0.0)
    for g in range(2):
        for k in range(4):
            s = slice(32 * k, 32 * k + 32)
            nc.vector.tensor_copy(out=bdctx[s, g, s], in_=ctx_sb[s, g, s])

    # ---- residual source in r-layout (f32), loaded late ----
    xr = bigs.tile([P, 2, N], F32, tag="xr")
    x_r = x_e.rearrange("(cc p qq) c -> p cc (qq c)", p=P, qq=Q16)
    for cc in range(2):
        for h in range(2):
            nc.gpsimd.dma_start(
                out=xr[:, cc, 2048 * h : 2048 * h + 2048],
                in_=x_r[:, cc, 2048 * h : 2048 * h + 2048],
            )

    out_r = out_e.rearrange("(cc p qq) c -> p cc (qq c)", p=P, qq=Q16)

    # ---- phase 2b: stream 8 blocks of 2 q-columns (512 output cols each) ----
    for qq in range(8):
        sqr = p2pool.tile([P, 2, 2, C], F32, tag="sqr")  # (g, qi, c)
        agg = p2pool.tile([P, 2, 2, C], BF16, tag="agg")  # (qi, rc, c)
        for g in range(2):
            sqp = psumA.tile([P, 2, C], F32, tag="mm", name="sqp")
            nc.tensor.matmul(
                sqp, lhsT=blockones, rhs=expQall[:, 2 * qq : 2 * qq + 2, g, :],
                start=True, stop=True,
            )
            attp = psumA.tile([P, 2, C], F32, tag="mm", name="attp")
            nc.tensor.matmul(
                attp, lhsT=bdctx[:, g, :], rhs=expQall[:, 2 * qq : 2 * qq + 2, g, :],
                start=True, stop=True,
            )
            nc.vector.reciprocal_approx_fast(out=sqr[:, g, :, :], in_=sqp)
            nc.vector.tensor_mul(out=agg[:, :, g, :], in0=attp, in1=sqr[:, g, :, :])
        for oc in range(2):
            pp = psumA.tile([P, 2, C], F32, tag="mm", name="pp")
            for rc2 in range(2):
                nc.tensor.matmul(
                    pp,
                    lhsT=wts["p"][:, rc2, 128 * oc : 128 * oc + 128],
                    rhs=agg[:, :, rc2, :],
                    start=(rc2 == 0),
                    stop=(rc2 == 1),
                )
            y = ypool.tile([P, 2, C], F32, tag="y")
            nc.vector.tensor_scalar(
                out=y, in0=pp, scalar1=bp_sb[:, oc : oc + 1],
                scalar2=None, op0=mybir.AluOpType.add,
            )
            nc.gpsimd.tensor_add(
                out=y,
                in0=y,
                in1=xr[:, oc, 512 * qq : 512 * qq + 512].rearrange(
                    "p (qi c) -> p qi c", qi=2
                ),
            )
            nc.scalar.dma_start(out=out_r[:, oc, 512 * qq : 512 * qq + 512], in_=y)

    ctx.close()


def _get_nc():
    if "nc" not in _CACHE:
        _CACHE["nc"] = _build_program()
    return _CACHE["nc"]


def kernel(**inputs):
    x = np.ascontiguousarray(np.asarray(inputs["x"], dtype=np.float32))
    B = x.shape[0]
    ws = {k: np.ascontiguousarray(np.asarray(inputs[k], dtype=np.float32))
          for k in ("Wq", "Wk", "Wv", "Wp", "bp")}

    nc = _get_nc()
    in_maps = [dict(x=x[b], **ws) for b in range(B)]
    res = run_bass_kernel_spmd(nc, in_maps, list(range(NCORES)))
    out = np.stack([res.results[b]["out"] for b in range(B)], axis=0)
    return out.astype(np.float32)


# revision 17
# speedup vs baseline: 1.0064x; 1.0064x over previous
"""Trainium2 Bass kernel for nn_Attention_61229053772048 (dual-softmax linear attention).

Sharding: data-parallel over batch B=8 across 8 NeuronCores (one batch per core,
no collectives). Each core computes, for its batch x_b (4096, 256):

  K = x Wk^T, Q = x Wq^T, V = x Wv^T              (raw-reshape semantics:
  r-layout M_r (256, 4096) = row-major view of M (4096, 256))
  key softmax over the 4096 axis of K_r, query softmax over 32-row head groups
  of Q_r, context = Ksm_h @ V_h^T per head (32x32), attended = ctx^T @ Qsm,
  proj = Wp @ attended + bp + x_r, out = raw reshape back to (4096, 256).

v3 layout choices (from profile iteration):
  - all big matmuls in bf16 (fp32 matmuls run LOW_HIGH = 2x passes)
  - x is cast to bf16, bounced through DRAM, and transposed via the DMA XBAR
    into a q-major xT2 (128c, 2cc, 16q, 256r) so every matmul operand is
    contiguous (strided rhs reads measured 6x slower)
  - K/V projections as N=512 matmuls; Sk folded into the context matmul via a
    ones column embedded in the V tile (rhs width 129)
  - phase 2 streams 8 blocks of 2 q-columns: Q proj -> exp -> Sq/attended
    (block-diag lhsT, N=512) -> fast-reciprocal divide -> out proj -> +bias,
    +residual (gpsimd) -> DMA out
"""

import sys

sys.path.insert(0, "/opt/trn_rl_repo")

import numpy as np

import concourse.bass as bass
import concourse.bacc as bacc_mod
import concourse.tile as tile
from concourse import mybir
from concourse.bass_utils import run_bass_kernel_spmd
from concourse.masks import make_identity

F32 = mybir.dt.float32
BF16 = mybir.dt.bfloat16
Exp = mybir.ActivationFunctionType.Exp

N, C, P = 4096, 256, 128
NH, HD, Q16 = 8, 32, 16
NCORES = 8

_CACHE = {}


def _build_program():
    nc = bacc_mod.Bacc(None, target_bir_lowering=False, debug=False)
    x_e = nc.declare_dram_parameter("x", [N, C], F32, isOutput=False)
    wq_e = nc.declare_dram_parameter("Wq", [C, C], F32, isOutput=False)
    wk_e = nc.declare_dram_parameter("Wk", [C, C], F32, isOutput=False)
    wv_e = nc.declare_dram_parameter("Wv", [C, C], F32, isOutput=False)
    wp_e = nc.declare_dram_parameter("Wp", [C, C], F32, isOutput=False)
    bp_e = nc.declare_dram_parameter("bp", [C], F32, isOutput=False)
    out_e = nc.declare_dram_parameter("out", [N, C], F32, isOutput=True)

    with tile.TileContext(nc) as tc:
        _body(tc, x_e, wq_e, wk_e, wv_e, wp_e, bp_e, out_e)
    nc.compile()
    return nc


def _body(tc, x_e, wq_e, wk_e, wv_e, wp_e, bp_e, out_e):
    nc = tc.nc
    from contextlib import ExitStack

    ctx = ExitStack()
    consts = ctx.enter_context(tc.tile_pool(name="consts", bufs=1))
    wstage = ctx.enter_context(tc.tile_pool(name="wstage", bufs=2))
    xstage = ctx.enter_context(tc.tile_pool(name="xstage", bufs=3))
    bigs = ctx.enter_context(tc.tile_pool(name="bigs", bufs=1))
    p2pool = ctx.enter_context(tc.tile_pool(name="p2", bufs=3))
    ypool = ctx.enter_context(tc.tile_pool(name="y", bufs=4))
    dram = ctx.enter_context(tc.tile_pool(name="dram", bufs=1, space="DRAM"))
    psumA = ctx.enter_context(tc.tile_pool(name="psumA", bufs=4, space="PSUM"))
    psumT = ctx.enter_context(tc.tile_pool(name="psumT", bufs=2, space="PSUM"))

    # ---- constants ----
    identity = consts.tile([P, P], BF16)
    make_identity(nc, identity)

    blockones = consts.tile([P, P], BF16)
    nc.vector.memset(blockones, 0.0)
    for k in range(4):
        nc.vector.memset(blockones[32 * k : 32 * k + 32, 32 * k : 32 * k + 32], 1.0)

    bp_sb = consts.tile([P, 2], F32)
    nc.sync.dma_start(out=bp_sb, in_=bp_e.rearrange("(cc p) -> p cc", p=P))

    # ---- weight transposes: wt[p, cc, o] = W[o, 128*cc + p]  (bf16) ----
    wts = {}
    for name, w_e in (("q", wq_e), ("k", wk_e), ("v", wv_e), ("p", wp_e)):
        wn = wstage.tile([P, 2, C], F32, tag="wn")
        w_v = w_e.rearrange("(oc p) c -> p oc c", p=P)
        for oc in range(2):
            nc.sync.dma_start(out=wn[:, oc, :], in_=w_v[:, oc, :])
        wnb = wstage.tile([P, 2, C], BF16, tag="wnb")
        for oc in range(2):
            nc.vector.tensor_copy(out=wnb[:, oc, :], in_=wn[:, oc, :])
        wt = consts.tile([P, 2, C], BF16, tag=f"wt_{name}")
        for cc in range(2):
            for oc in range(2):
                ps = psumT.tile([P, P], BF16, tag="tw")
                nc.tensor.transpose(ps, wnb[:, oc, 128 * cc : 128 * cc + 128], identity)
                nc.vector.tensor_copy(out=wt[:, cc, 128 * oc : 128 * oc + 128], in_=ps)
        wts[name] = wt

    # ---- x -> bf16 -> DRAM bounce -> XBAR transpose into q-major xT2 ----
    # xT2[p, cc, q, r] = x[16r + q, 128cc + p]  (bf16)
    xbf = dram.tile([N, C], BF16)
    x_nt = x_e.rearrange("(nt p) c -> p nt c", p=P)  # (128, 32, 256)
    xbf_nt = xbf.rearrange("(nt p) c -> p nt c", p=P)
    for grp in range(8):
        xs = xstage.tile([P, 4, C], F32, tag="xs")
        nc.scalar.dma_start(out=xs, in_=x_nt[:, 4 * grp : 4 * grp + 4, :])
        xsb = xstage.tile([P, 4, C], BF16, tag="xsb")
        nc.vector.tensor_copy(out=xsb, in_=xs)
        nc.gpsimd.dma_start(out=xbf_nt[:, 4 * grp : 4 * grp + 4, :], in_=xsb)

    xT2 = bigs.tile([P, 2, Q16, C], BF16, tag="xT2")
    xbf_q = xbf.rearrange("(r q) c -> q r c", q=Q16)  # row 16r+q
    for q in range(Q16):
        for cc in range(2):
            nc.sync.dma_start_transpose(
                xT2[:, cc, q, :], xbf_q[q, :, 128 * cc : 128 * cc + 128]
            )

    # ---- K and V in n-part layout: chunk t = 2q + cj holds partitions c-half ----
    # dst[p, t, r] = sum_{c'} W[128*cj + p, c'] * x[16r + q, c']
    # Vb free layout per chunk: [g0 V(128) | 1.0 | g1 V(128) | 1.0] (129*2)
    # Q projections interleaved here so PE has work while K/V epilogues drain.
    expK = bigs.tile([P, 32, C], BF16, tag="expK")
    Vb = bigs.tile([P, 32, 258], BF16, tag="Vb")
    Vb_v = Vb.rearrange("p t (g x) -> p t g x", g=2)
    nc.vector.memset(Vb_v[:, :, :, 128], 1.0)
    expQall = bigs.tile([P, Q16, 2, C], BF16, tag="expQall")
    for cj in range(2):
        for qp in range(8):
            for wt, do_exp in ((wts["k"], True), (wts["v"], False)):
                ps = psumA.tile([P, 2, C], F32, tag="mm", name="kvps")
                for cp in range(2):
                    nc.tensor.matmul(
                        ps,
                        lhsT=wt[:, cp, 128 * cj : 128 * cj + 128],
                        rhs=xT2[:, cp, 2 * qp : 2 * qp + 2, :],
                        start=(cp == 0),
                        stop=(cp == 1),
                    )
                t0 = 4 * qp + cj  # chunks t0 and t0 + 2
                if do_exp:
                    nc.scalar.activation(
                        out=expK[:, t0 : t0 + 3 : 2, :], in_=ps, func=Exp
                    )
                else:
                    nc.vector.tensor_copy(
                        out=Vb_v[:, t0 : t0 + 3 : 2, :, 0:128],
                        in_=ps.rearrange("p two (g e) -> p two g e", g=2),
                    )
            for qi in range(2):
                q = 2 * qp + qi
                rc = cj
                qp_full = psumA.tile([P, 2, C], F32, tag="mm", name="qp_full")
                qp_ps = qp_full[:, 0, :]
                for cp in range(2):
                    nc.tensor.matmul(
                        qp_ps,
                        lhsT=xT2[:, cp, q, 128 * rc : 128 * rc + 128],
                        rhs=wts["q"][:, cp, :],
                        start=(cp == 0),
                        stop=(cp == 1),
                    )
                nc.scalar.activation(out=expQall[:, q, rc, :], in_=qp_ps, func=Exp)

    # ---- context + Sk (ones column) per 128-r-group, contraction over n ----
    # ctxp[d, e] = sum_n expK[n, 128g+d] * V[n, 128g+e];  col 128 = Sk
    ctx_sb = consts.tile([P, 2, P], F32, tag="ctx")
    recip_sk = consts.tile([P, 2], F32, tag="rsk")
    for g in range(2):
        ctxp = psumT.tile([P, 132], F32, tag="tc")
        for t in range(32):
            nc.tensor.matmul(
                ctxp[:, :129],
                lhsT=expK[:, t, 128 * g : 128 * g + 128],
                rhs=Vb[:, t, 129 * g : 129 * g + 129],
                start=(t == 0),
                stop=(t == 31),
            )
        nc.vector.reciprocal_approx_fast(
            out=recip_sk[:, g : g + 1], in_=ctxp[:, 128:129]
        )
        nc.vector.tensor_scalar_mul(
            out=ctx_sb[:, g, :], in0=ctxp[:, 0:128], scalar1=recip_sk[:, g : g + 1]
        )

    # ---- block-diagonal context for the attended matmul ----
    bdctx = consts.tile([P, 2, P], BF16, tag="bdctx")
    nc.vector.memset(bdctx, 0.0)
    for g in range(2):
        for k in range(4):
            s = slice(32 * k, 32 * k + 32)
            nc.vector.tensor_copy(out=bdctx[s, g, s], in_=ctx_sb[s, g, s])

    # ---- residual source in r-layout (f32), loaded late ----
    xr = bigs.tile([P, 2, N], F32, tag="xr")
    x_r = x_e.rearrange("(cc p qq) c -> p cc (qq c)", p=P, qq=Q16)
    for cc in range(2):
        for h in range(4):
            nc.gpsimd.dma_start(
                out=xr[:, cc, 1024 * h : 1024 * h + 1024],
                in_=x_r[:, cc, 1024 * h : 1024 * h + 1024],
            )

    out_r = out_e.rearrange("(cc p qq) c -> p cc (qq c)", p=P, qq=Q16)

    # ---- phase 2b: stream 8 blocks of 2 q-columns (512 output cols each) ----
    for qq in range(8):
        sqr = p2pool.tile([P, 2, 2, C], F32, tag="sqr")  # (g, qi, c)
        agg = p2pool.tile([P, 2, 2, C], BF16, tag="agg")  # (qi, rc, c)
        for g in range(2):
            sqp = psumA.tile([P, 2, C], F32, tag="mm", name="sqp")
            nc.tensor.matmul(
                sqp, lhsT=blockones, rhs=expQall[:, 2 * qq : 2 * qq + 2, g, :],
                start=True, stop=True,
            )
            attp = psumA.tile([P, 2, C], F32, tag="mm", name="attp")
            nc.tensor.matmul(
                attp, lhsT=bdctx[:, g, :], rhs=expQall[:, 2 * qq : 2 * qq + 2, g, :],
                start=True, stop=True,
            )
            nc.vector.reciprocal_approx_fast(out=sqr[:, g, :, :], in_=sqp)
            nc.vector.tensor_mul(out=agg[:, :, g, :], in0=attp, in1=sqr[:, g, :, :])
        for oc in range(2):
            pp = psumA.tile([P, 2, C], F32, tag="mm", name="pp")
            for rc2 in range(2):
                nc.tensor.matmul(
                    pp,
                    lhsT=wts["p"][:, rc2, 128 * oc : 128 * oc + 128],
                    rhs=agg[:, :, rc2, :],
                    start=(rc2 == 0),
                    stop=(rc2 == 1),
                )
            y = ypool.tile([P, 2, C], F32, tag="y")
            nc.scalar.activation(
                out=y, in_=pp, func=mybir.ActivationFunctionType.Identity,
                bias=bp_sb[:, oc : oc + 1],
            )
            nc.gpsimd.tensor_add(
                out=y,
                in0=y,
                in1=xr[:, oc, 512 * qq : 512 * qq + 512].rearrange(
                    "p (qi c) -> p qi c", qi=2
                ),
            )
            nc.scalar.dma_start(out=out_r[:, oc, 512 * qq : 512 * qq + 512], in_=y)

    ctx.close()


def _get_nc():
    if "nc" not in _CACHE:
        _CACHE["nc"] = _build_program()
    return _CACHE["nc"]


def kernel(**inputs):
    x = np.ascontiguousarray(np.asarray(inputs["x"], dtype=np.float32))
    B = x.shape[0]
    ws = {k: np.ascontiguousarray(np.asarray(inputs[k], dtype=np.float32))
          for k in ("Wq", "Wk", "Wv", "Wp", "bp")}

    nc = _get_nc()
    in_maps = [dict(x=x[b], **ws) for b in range(B)]
    res = run_bass_kernel_spmd(nc, in_maps, list(range(NCORES)))
    out = np.stack([res.results[b]["out"] for b in range(B)], axis=0)
    return out.astype(np.float32)


# revision 19
# speedup vs baseline: 1.0686x; 1.0618x over previous
"""Trainium2 Bass kernel for nn_Attention_61229053772048 (dual-softmax linear attention).

Sharding: data-parallel over batch B=8 across 8 NeuronCores (one batch per core,
no collectives). Each core computes, for its batch x_b (4096, 256):

  K = x Wk^T, Q = x Wq^T, V = x Wv^T              (raw-reshape semantics:
  r-layout M_r (256, 4096) = row-major view of M (4096, 256))
  key softmax over the 4096 axis of K_r, query softmax over 32-row head groups
  of Q_r, context = Ksm_h @ V_h^T per head (32x32), attended = ctx^T @ Qsm,
  proj = Wp @ attended + bp + x_r, out = raw reshape back to (4096, 256).

v3 layout choices (from profile iteration):
  - all big matmuls in bf16 (fp32 matmuls run LOW_HIGH = 2x passes)
  - x is cast to bf16, bounced through DRAM, and transposed via the DMA XBAR
    into a q-major xT2 (128c, 2cc, 16q, 256r) so every matmul operand is
    contiguous (strided rhs reads measured 6x slower)
  - K/V projections as N=512 matmuls; Sk folded into the context matmul via a
    ones column embedded in the V tile (rhs width 129)
  - phase 2 streams 8 blocks of 2 q-columns: Q proj -> exp -> Sq/attended
    (block-diag lhsT, N=512) -> fast-reciprocal divide -> out proj -> +bias,
    +residual (gpsimd) -> DMA out
"""

import sys

sys.path.insert(0, "/opt/trn_rl_repo")

import numpy as np

import concourse.bass as bass
import concourse.bacc as bacc_mod
import concourse.tile as tile
from concourse import mybir
from concourse.bass_utils import run_bass_kernel_spmd
from concourse.masks import make_identity

F32 = mybir.dt.float32
BF16 = mybir.dt.bfloat16
Exp = mybir.ActivationFunctionType.Exp

N, C, P = 4096, 256, 128
NH, HD, Q16 = 8, 32, 16
NCORES = 8

_CACHE = {}


def _build_program():
    nc = bacc_mod.Bacc(None, target_bir_lowering=False, debug=False)
    x_e = nc.declare_dram_parameter("x", [N, C], F32, isOutput=False)
    wq_e = nc.declare_dram_parameter("Wq", [C, C], F32, isOutput=False)
    wk_e = nc.declare_dram_parameter("Wk", [C, C], F32, isOutput=False)
    wv_e = nc.declare_dram_parameter("Wv", [C, C], F32, isOutput=False)
    wp_e = nc.declare_dram_parameter("Wp", [C, C], F32, isOutput=False)
    bp_e = nc.declare_dram_parameter("bp", [C], F32, isOutput=False)
    out_e = nc.declare_dram_parameter("out", [N, C], F32, isOutput=True)

    with tile.TileContext(nc) as tc:
        _body(tc, x_e, wq_e, wk_e, wv_e, wp_e, bp_e, out_e)
    nc.compile()
    return nc


def _body(tc, x_e, wq_e, wk_e, wv_e, wp_e, bp_e, out_e):
    nc = tc.nc
    from contextlib import ExitStack

    ctx = ExitStack()
    consts = ctx.enter_context(tc.tile_pool(name="consts", bufs=1))
    wstage = ctx.enter_context(tc.tile_pool(name="wstage", bufs=2))
    xstage = ctx.enter_context(tc.tile_pool(name="xstage", bufs=3))
    bigs = ctx.enter_context(tc.tile_pool(name="bigs", bufs=1))
    p2pool = ctx.enter_context(tc.tile_pool(name="p2", bufs=3))
    ypool = ctx.enter_context(tc.tile_pool(name="y", bufs=4))
    dram = ctx.enter_context(tc.tile_pool(name="dram", bufs=1, space="DRAM"))
    psumA = ctx.enter_context(tc.tile_pool(name="psumA", bufs=4, space="PSUM"))
    psumT = ctx.enter_context(tc.tile_pool(name="psumT", bufs=2, space="PSUM"))

    # ---- constants ----
    identity = consts.tile([P, P], BF16)
    make_identity(nc, identity)

    blockones = consts.tile([P, P], BF16)
    nc.vector.memset(blockones, 0.0)
    for k in range(4):
        nc.vector.memset(blockones[32 * k : 32 * k + 32, 32 * k : 32 * k + 32], 1.0)

    bp_sb = consts.tile([P, 2], F32)
    nc.sync.dma_start(out=bp_sb, in_=bp_e.rearrange("(cc p) -> p cc", p=P))

    # ---- weight transposes: wt[p, cc, o] = W[o, 128*cc + p]  (bf16) ----
    wts = {}
    for name, w_e in (("q", wq_e), ("k", wk_e), ("v", wv_e), ("p", wp_e)):
        wn = wstage.tile([P, 2, C], F32, tag="wn")
        w_v = w_e.rearrange("(oc p) c -> p oc c", p=P)
        for oc in range(2):
            nc.sync.dma_start(out=wn[:, oc, :], in_=w_v[:, oc, :])
        wnb = wstage.tile([P, 2, C], BF16, tag="wnb")
        for oc in range(2):
            nc.vector.tensor_copy(out=wnb[:, oc, :], in_=wn[:, oc, :])
        wt = consts.tile([P, 2, C], BF16, tag=f"wt_{name}")
        for cc in range(2):
            for oc in range(2):
                ps = psumT.tile([P, P], BF16, tag="tw")
                nc.tensor.transpose(ps, wnb[:, oc, 128 * cc : 128 * cc + 128], identity)
                nc.vector.tensor_copy(out=wt[:, cc, 128 * oc : 128 * oc + 128], in_=ps)
        wts[name] = wt

    # ---- x -> bf16 -> PE transpose into q-major xT2 ----
    # xT2[p, cc, q, r] = x[16r + q, 128cc + p]  (bf16)
    # transpose of token-block nt scatters: tp[:, j] -> xT2[:, cj, j%16, 8nt + j//16]
    xT2 = bigs.tile([P, 2, Q16, C], BF16, tag="xT2")
    x_nt = x_e.rearrange("(nt p) c -> p nt c", p=P)  # (128, 32, 256)
    for grp in range(8):
        xs = xstage.tile([P, 4, C], F32, tag="xs")
        nc.sync.dma_start(out=xs, in_=x_nt[:, 4 * grp : 4 * grp + 4, :])
        xsb = xstage.tile([P, 4, C], BF16, tag="xsb")
        nc.vector.tensor_copy(out=xsb, in_=xs)
        for t4 in range(4):
            nt = 4 * grp + t4
            for cj in range(2):
                tp = psumT.tile([P, P], BF16, tag="tw")
                nc.tensor.transpose(tp, xsb[:, t4, 128 * cj : 128 * cj + 128], identity)
                nc.vector.tensor_copy(
                    out=xT2[:, cj, :, 8 * nt : 8 * nt + 8].rearrange("p q r -> p r q"),
                    in_=tp.rearrange("p (r q) -> p r q", q=Q16),
                )

    # ---- K and V in n-part layout: chunk t = 2q + cj holds partitions c-half ----
    # dst[p, t, r] = sum_{c'} W[128*cj + p, c'] * x[16r + q, c']
    # Vb free layout per chunk: [g0 V(128) | 1.0 | g1 V(128) | 1.0] (129*2)
    # Q projections interleaved here so PE has work while K/V epilogues drain.
    expK = bigs.tile([P, 32, C], BF16, tag="expK")
    Vb = bigs.tile([P, 32, 258], BF16, tag="Vb")
    Vb_v = Vb.rearrange("p t (g x) -> p t g x", g=2)
    nc.vector.memset(Vb_v[:, :, :, 128], 1.0)
    expQall = bigs.tile([P, Q16, 2, C], BF16, tag="expQall")
    for cj in range(2):
        for qp in range(8):
            for wt, do_exp in ((wts["k"], True), (wts["v"], False)):
                ps = psumA.tile([P, 2, C], F32, tag="mm", name="kvps")
                for cp in range(2):
                    nc.tensor.matmul(
                        ps,
                        lhsT=wt[:, cp, 128 * cj : 128 * cj + 128],
                        rhs=xT2[:, cp, 2 * qp : 2 * qp + 2, :],
                        start=(cp == 0),
                        stop=(cp == 1),
                    )
                t0 = 4 * qp + cj  # chunks t0 and t0 + 2
                if do_exp:
                    nc.scalar.activation(
                        out=expK[:, t0 : t0 + 3 : 2, :], in_=ps, func=Exp
                    )
                else:
                    nc.vector.tensor_copy(
                        out=Vb_v[:, t0 : t0 + 3 : 2, :, 0:128],
                        in_=ps.rearrange("p two (g e) -> p two g e", g=2),
                    )
            for qi in range(2):
                q = 2 * qp + qi
                rc = cj
                qp_full = psumA.tile([P, 2, C], F32, tag="mm", name="qp_full")
                qp_ps = qp_full[:, 0, :]
                for cp in range(2):
                    nc.tensor.matmul(
                        qp_ps,
                        lhsT=xT2[:, cp, q, 128 * rc : 128 * rc + 128],
                        rhs=wts["q"][:, cp, :],
                        start=(cp == 0),
                        stop=(cp == 1),
                    )
                nc.scalar.activation(out=expQall[:, q, rc, :], in_=qp_ps, func=Exp)

    # ---- context + Sk (ones column) per 128-r-group, contraction over n ----
    # ctxp[d, e] = sum_n expK[n, 128g+d] * V[n, 128g+e];  col 128 = Sk
    ctx_sb = consts.tile([P, 2, P], F32, tag="ctx")
    recip_sk = consts.tile([P, 2], F32, tag="rsk")
    for g in range(2):
        ctxp = psumT.tile([P, 132], F32, tag="tc")
        for t in range(32):
            nc.tensor.matmul(
                ctxp[:, :129],
                lhsT=expK[:, t, 128 * g : 128 * g + 128],
                rhs=Vb[:, t, 129 * g : 129 * g + 129],
                start=(t == 0),
                stop=(t == 31),
            )
        nc.vector.reciprocal_approx_fast(
            out=recip_sk[:, g : g + 1], in_=ctxp[:, 128:129]
        )
        nc.vector.tensor_scalar_mul(
            out=ctx_sb[:, g, :], in0=ctxp[:, 0:128], scalar1=recip_sk[:, g : g + 1]
        )

    # ---- block-diagonal context for the attended matmul ----
    bdctx = consts.tile([P, 2, P], BF16, tag="bdctx")
    nc.vector.memset(bdctx, 0.0)
    for g in range(2):
        for k in range(4):
            s = slice(32 * k, 32 * k + 32)
            nc.vector.tensor_copy(out=bdctx[s, g, s], in_=ctx_sb[s, g, s])

    # ---- residual source in r-layout (f32), loaded late ----
    xr = bigs.tile([P, 2, N], F32, tag="xr")
    x_r = x_e.rearrange("(cc p qq) c -> p cc (qq c)", p=P, qq=Q16)
    for cc in range(2):
        for h in range(4):
            nc.gpsimd.dma_start(
                out=xr[:, cc, 1024 * h : 1024 * h + 1024],
                in_=x_r[:, cc, 1024 * h : 1024 * h + 1024],
            )

    out_r = out_e.rearrange("(cc p qq) c -> p cc (qq c)", p=P, qq=Q16)

    # ---- phase 2b: stream 8 blocks of 2 q-columns (512 output cols each) ----
    for qq in range(8):
        sqr = p2pool.tile([P, 2, 2, C], F32, tag="sqr")  # (g, qi, c)
        agg = p2pool.tile([P, 2, 2, C], BF16, tag="agg")  # (qi, rc, c)
        for g in range(2):
            sqp = psumA.tile([P, 2, C], F32, tag="mm", name="sqp")
            nc.tensor.matmul(
                sqp, lhsT=blockones, rhs=expQall[:, 2 * qq : 2 * qq + 2, g, :],
                start=True, stop=True,
            )
            attp = psumA.tile([P, 2, C], F32, tag="mm", name="attp")
            nc.tensor.matmul(
                attp, lhsT=bdctx[:, g, :], rhs=expQall[:, 2 * qq : 2 * qq + 2, g, :],
                start=True, stop=True,
            )
            nc.vector.reciprocal_approx_fast(out=sqr[:, g, :, :], in_=sqp)
            nc.vector.tensor_mul(out=agg[:, :, g, :], in0=attp, in1=sqr[:, g, :, :])
        for oc in range(2):
            pp = psumA.tile([P, 2, C], F32, tag="mm", name="pp")
            for rc2 in range(2):
                nc.tensor.matmul(
                    pp,
                    lhsT=wts["p"][:, rc2, 128 * oc : 128 * oc + 128],
                    rhs=agg[:, :, rc2, :],
                    start=(rc2 == 0),
                    stop=(rc2 == 1),
                )
            y = ypool.tile([P, 2, C], F32, tag="y")
            nc.scalar.activation(
                out=y, in_=pp, func=mybir.ActivationFunctionType.Identity,
                bias=bp_sb[:, oc : oc + 1],
            )
            nc.gpsimd.tensor_add(
                out=y,
                in0=y,
                in1=xr[:, oc, 512 * qq : 512 * qq + 512].rearrange(
                    "p (qi c) -> p qi c", qi=2
                ),
            )
            nc.scalar.dma_start(out=out_r[:, oc, 512 * qq : 512 * qq + 512], in_=y)

    ctx.close()


def _get_nc():
    if "nc" not in _CACHE:
        _CACHE["nc"] = _build_program()
    return _CACHE["nc"]


def kernel(**inputs):
    x = np.ascontiguousarray(np.asarray(inputs["x"], dtype=np.float32))
    B = x.shape[0]
    ws = {k: np.ascontiguousarray(np.asarray(inputs[k], dtype=np.float32))
          for k in ("Wq", "Wk", "Wv", "Wp", "bp")}

    nc = _get_nc()
    in_maps = [dict(x=x[b], **ws) for b in range(B)]
    res = run_bass_kernel_spmd(nc, in_maps, list(range(NCORES)))
    out = np.stack([res.results[b]["out"] for b in range(B)], axis=0)
    return out.astype(np.float32)


# revision 20
# speedup vs baseline: 1.3023x; 1.2187x over previous
"""Trainium2 Bass kernel for nn_Attention_61229053772048 (dual-softmax linear attention).

Sharding: data-parallel over batch B=8 across 8 NeuronCores (one batch per core,
no collectives). Each core computes, for its batch x_b (4096, 256):

  K = x Wk^T, Q = x Wq^T, V = x Wv^T              (raw-reshape semantics:
  r-layout M_r (256, 4096) = row-major view of M (4096, 256))
  key softmax over the 4096 axis of K_r, query softmax over 32-row head groups
  of Q_r, context = Ksm_h @ V_h^T per head (32x32), attended = ctx^T @ Qsm,
  proj = Wp @ attended + bp + x_r, out = raw reshape back to (4096, 256).

v3 layout choices (from profile iteration):
  - all big matmuls in bf16 (fp32 matmuls run LOW_HIGH = 2x passes)
  - x is cast to bf16, bounced through DRAM, and transposed via the DMA XBAR
    into a q-major xT2 (128c, 2cc, 16q, 256r) so every matmul operand is
    contiguous (strided rhs reads measured 6x slower)
  - K/V projections as N=512 matmuls; Sk folded into the context matmul via a
    ones column embedded in the V tile (rhs width 129)
  - phase 2 streams 8 blocks of 2 q-columns: Q proj -> exp -> Sq/attended
    (block-diag lhsT, N=512) -> fast-reciprocal divide -> out proj -> +bias,
    +residual (gpsimd) -> DMA out
"""

import sys

sys.path.insert(0, "/opt/trn_rl_repo")

import numpy as np

import concourse.bass as bass
import concourse.bacc as bacc_mod
import concourse.tile as tile
from concourse import mybir
from concourse.bass_utils import run_bass_kernel_spmd
from concourse.masks import make_identity

F32 = mybir.dt.float32
BF16 = mybir.dt.bfloat16
Exp = mybir.ActivationFunctionType.Exp

N, C, P = 4096, 256, 128
NH, HD, Q16 = 8, 32, 16
NCORES = 8

_CACHE = {}


def _build_program():
    nc = bacc_mod.Bacc(None, target_bir_lowering=False, debug=False)
    x_e = nc.declare_dram_parameter("x", [N, C], F32, isOutput=False)
    wq_e = nc.declare_dram_parameter("Wq", [C, C], F32, isOutput=False)
    wk_e = nc.declare_dram_parameter("Wk", [C, C], F32, isOutput=False)
    wv_e = nc.declare_dram_parameter("Wv", [C, C], F32, isOutput=False)
    wp_e = nc.declare_dram_parameter("Wp", [C, C], F32, isOutput=False)
    bp_e = nc.declare_dram_parameter("bp", [C], F32, isOutput=False)
    out_e = nc.declare_dram_parameter("out", [N, C], F32, isOutput=True)

    with tile.TileContext(nc) as tc:
        _body(tc, x_e, wq_e, wk_e, wv_e, wp_e, bp_e, out_e)
    nc.compile()
    return nc


def _body(tc, x_e, wq_e, wk_e, wv_e, wp_e, bp_e, out_e):
    nc = tc.nc
    from contextlib import ExitStack

    ctx = ExitStack()
    consts = ctx.enter_context(tc.tile_pool(name="consts", bufs=1))
    wstage = ctx.enter_context(tc.tile_pool(name="wstage", bufs=2))
    xstage = ctx.enter_context(tc.tile_pool(name="xstage", bufs=3))
    bigs = ctx.enter_context(tc.tile_pool(name="bigs", bufs=1))
    p2pool = ctx.enter_context(tc.tile_pool(name="p2", bufs=3))
    ypool = ctx.enter_context(tc.tile_pool(name="y", bufs=4))
    dram = ctx.enter_context(tc.tile_pool(name="dram", bufs=1, space="DRAM"))
    psumA = ctx.enter_context(tc.tile_pool(name="psumA", bufs=4, space="PSUM"))
    psumT = ctx.enter_context(tc.tile_pool(name="psumT", bufs=2, space="PSUM"))

    # ---- constants ----
    identity = consts.tile([P, P], BF16)
    make_identity(nc, identity)

    blockones = consts.tile([P, P], BF16)
    nc.vector.memset(blockones, 0.0)
    for k in range(4):
        nc.vector.memset(blockones[32 * k : 32 * k + 32, 32 * k : 32 * k + 32], 1.0)

    bp_sb = consts.tile([P, 2], F32)
    nc.sync.dma_start(out=bp_sb, in_=bp_e.rearrange("(cc p) -> p cc", p=P))

    # ---- weight transposes: wt[p, cc, o] = W[o, 128*cc + p]  (bf16) ----
    wts = {}
    for name, w_e in (("q", wq_e), ("k", wk_e), ("v", wv_e), ("p", wp_e)):
        wn = wstage.tile([P, 2, C], F32, tag="wn")
        w_v = w_e.rearrange("(oc p) c -> p oc c", p=P)
        for oc in range(2):
            nc.sync.dma_start(out=wn[:, oc, :], in_=w_v[:, oc, :])
        wnb = wstage.tile([P, 2, C], BF16, tag="wnb")
        for oc in range(2):
            nc.vector.tensor_copy(out=wnb[:, oc, :], in_=wn[:, oc, :])
        wt = consts.tile([P, 2, C], BF16, tag=f"wt_{name}")
        for cc in range(2):
            for oc in range(2):
                ps = psumT.tile([P, P], BF16, tag="tw")
                nc.tensor.transpose(ps, wnb[:, oc, 128 * cc : 128 * cc + 128], identity)
                nc.vector.tensor_copy(out=wt[:, cc, 128 * oc : 128 * oc + 128], in_=ps)
        wts[name] = wt

    # ---- x loaded q-gathered from HBM (strided rows), PE-transposed per tile ----
    # xT2[p, cc, q, r] = x[16r + q, 128cc + p]  (bf16); all on-chip copies contiguous.
    # K/V/Q matmuls interleaved per q-pair so PE pipelines with the DMA/cast feed.
    xT2 = bigs.tile([P, 2, Q16, C], BF16, tag="xT2")
    xq_v = x_e.rearrange("(r q) c -> q r c", q=Q16)
    expK = bigs.tile([P, 32, C], BF16, tag="expK")
    Vb = bigs.tile([P, 32, 258], BF16, tag="Vb")
    Vb_v = Vb.rearrange("p t (g x) -> p t g x", g=2)
    nc.vector.memset(Vb_v[:, :, :, 128], 1.0)
    expQall = bigs.tile([P, Q16, 2, C], BF16, tag="expQall")

    for qp in range(8):
        for qi in range(2):
            q = 2 * qp + qi
            for rh in range(2):
                xs = xstage.tile([P, C], F32, tag="xs")
                nc.sync.dma_start(out=xs, in_=xq_v[q, 128 * rh : 128 * rh + 128, :])
                xsb = xstage.tile([P, C], BF16, tag="xsb")
                nc.vector.tensor_copy(out=xsb, in_=xs)
                for cj in range(2):
                    tp = psumT.tile([P, P], BF16, tag="tw")
                    nc.tensor.transpose(
                        tp, xsb[:, 128 * cj : 128 * cj + 128], identity
                    )
                    nc.vector.tensor_copy(
                        out=xT2[:, cj, q, 128 * rh : 128 * rh + 128], in_=tp
                    )
        for cj in range(2):
            for wt, do_exp in ((wts["k"], True), (wts["v"], False)):
                ps = psumA.tile([P, 2, C], F32, tag="mm", name="kvps")
                for cp in range(2):
                    nc.tensor.matmul(
                        ps,
                        lhsT=wt[:, cp, 128 * cj : 128 * cj + 128],
                        rhs=xT2[:, cp, 2 * qp : 2 * qp + 2, :],
                        start=(cp == 0),
                        stop=(cp == 1),
                    )
                t0 = 4 * qp + cj  # chunks t0 and t0 + 2
                if do_exp:
                    nc.scalar.activation(
                        out=expK[:, t0 : t0 + 3 : 2, :], in_=ps, func=Exp
                    )
                else:
                    nc.vector.tensor_copy(
                        out=Vb_v[:, t0 : t0 + 3 : 2, :, 0:128],
                        in_=ps.rearrange("p two (g e) -> p two g e", g=2),
                    )
        for qi in range(2):
            q = 2 * qp + qi
            for rc in range(2):
                qp_full = psumA.tile([P, 2, C], F32, tag="mm", name="qp_full")
                qp_ps = qp_full[:, 0, :]
                for cp in range(2):
                    nc.tensor.matmul(
                        qp_ps,
                        lhsT=xT2[:, cp, q, 128 * rc : 128 * rc + 128],
                        rhs=wts["q"][:, cp, :],
                        start=(cp == 0),
                        stop=(cp == 1),
                    )
                nc.scalar.activation(out=expQall[:, q, rc, :], in_=qp_ps, func=Exp)

    # ---- context + Sk (ones column) per 128-r-group, contraction over n ----
    # ctxp[d, e] = sum_n expK[n, 128g+d] * V[n, 128g+e];  col 128 = Sk
    ctx_sb = consts.tile([P, 2, P], F32, tag="ctx")
    recip_sk = consts.tile([P, 2], F32, tag="rsk")
    for g in range(2):
        ctxp = psumT.tile([P, 132], F32, tag="tc")
        for t in range(32):
            nc.tensor.matmul(
                ctxp[:, :129],
                lhsT=expK[:, t, 128 * g : 128 * g + 128],
                rhs=Vb[:, t, 129 * g : 129 * g + 129],
                start=(t == 0),
                stop=(t == 31),
            )
        nc.vector.reciprocal_approx_fast(
            out=recip_sk[:, g : g + 1], in_=ctxp[:, 128:129]
        )
        nc.vector.tensor_scalar_mul(
            out=ctx_sb[:, g, :], in0=ctxp[:, 0:128], scalar1=recip_sk[:, g : g + 1]
        )

    # ---- block-diagonal context for the attended matmul ----
    bdctx = consts.tile([P, 2, P], BF16, tag="bdctx")
    nc.vector.memset(bdctx, 0.0)
    for g in range(2):
        for k in range(4):
            s = slice(32 * k, 32 * k + 32)
            nc.vector.tensor_copy(out=bdctx[s, g, s], in_=ctx_sb[s, g, s])

    # ---- residual source in r-layout (f32), loaded late ----
    xr = bigs.tile([P, 2, N], F32, tag="xr")
    x_r = x_e.rearrange("(cc p qq) c -> p cc (qq c)", p=P, qq=Q16)
    for cc in range(2):
        for h in range(4):
            nc.gpsimd.dma_start(
                out=xr[:, cc, 1024 * h : 1024 * h + 1024],
                in_=x_r[:, cc, 1024 * h : 1024 * h + 1024],
            )

    out_r = out_e.rearrange("(cc p qq) c -> p cc (qq c)", p=P, qq=Q16)

    # ---- phase 2b: stream 8 blocks of 2 q-columns (512 output cols each) ----
    for qq in range(8):
        sqr = p2pool.tile([P, 2, 2, C], F32, tag="sqr")  # (g, qi, c)
        agg = p2pool.tile([P, 2, 2, C], BF16, tag="agg")  # (qi, rc, c)
        for g in range(2):
            sqp = psumA.tile([P, 2, C], F32, tag="mm", name="sqp")
            nc.tensor.matmul(
                sqp, lhsT=blockones, rhs=expQall[:, 2 * qq : 2 * qq + 2, g, :],
                start=True, stop=True,
            )
            attp = psumA.tile([P, 2, C], F32, tag="mm", name="attp")
            nc.tensor.matmul(
                attp, lhsT=bdctx[:, g, :], rhs=expQall[:, 2 * qq : 2 * qq + 2, g, :],
                start=True, stop=True,
            )
            nc.vector.reciprocal_approx_fast(out=sqr[:, g, :, :], in_=sqp)
            nc.vector.tensor_mul(out=agg[:, :, g, :], in0=attp, in1=sqr[:, g, :, :])
        for oc in range(2):
            pp = psumA.tile([P, 2, C], F32, tag="mm", name="pp")
            for rc2 in range(2):
                nc.tensor.matmul(
                    pp,
                    lhsT=wts["p"][:, rc2, 128 * oc : 128 * oc + 128],
                    rhs=agg[:, :, rc2, :],
                    start=(rc2 == 0),
                    stop=(rc2 == 1),
                )
            y = ypool.tile([P, 2, C], F32, tag="y")
            nc.scalar.activation(
                out=y, in_=pp, func=mybir.ActivationFunctionType.Identity,
                bias=bp_sb[:, oc : oc + 1],
            )
            nc.gpsimd.tensor_add(
                out=y,
                in0=y,
                in1=xr[:, oc, 512 * qq : 512 * qq + 512].rearrange(
                    "p (qi c) -> p qi c", qi=2
                ),
            )
            nc.scalar.dma_start(out=out_r[:, oc, 512 * qq : 512 * qq + 512], in_=y)

    ctx.close()


def _get_nc():
    if "nc" not in _CACHE:
        _CACHE["nc"] = _build_program()
    return _CACHE["nc"]


def kernel(**inputs):
    x = np.ascontiguousarray(np.asarray(inputs["x"], dtype=np.float32))
    B = x.shape[0]
    ws = {k: np.ascontiguousarray(np.asarray(inputs[k], dtype=np.float32))
          for k in ("Wq", "Wk", "Wv", "Wp", "bp")}

    nc = _get_nc()
    in_maps = [dict(x=x[b], **ws) for b in range(B)]
    res = run_bass_kernel_spmd(nc, in_maps, list(range(NCORES)))
    out = np.stack([res.results[b]["out"] for b in range(B)], axis=0)
    return out.astype(np.float32)
